# revision 5
# baseline (speedup 1.0000x reference)
"""CARAFE upsample (nn_CARAFEPack) on 8 TRN2 NeuronCores via a Bass/Tile kernel.

Full inputs in, full output out. Sharding: core = (n, i): image n = core//2,
output-row parity i = core%2; each core produces out[n, :, i::2, :].

On-device per core: channel-compressor 1x1 conv and encoder 3x3 conv as
PSUM-accumulated f16 matmuls with fused bias (+exp) on the scalar engine;
softmax normalization via 0/1-matrix matmuls + reciprocal; CARAFE
reassembly with per-(j,tap) selector-matmul mask broadcasts, f16
evictions, merged DVE multiplies over shifted padded-x windows, and tap
accumulation in PSUM f32 via identity matmuls. Output f16.

The compiled executable (jax custom call wrapping the Bass NEFF, sharded
over 8 cores) is cached at module level so repeat kernel() calls skip
re-tracing.
"""
import sys

sys.path.insert(0, '/opt/trn_rl_repo')

import numpy as np

H = W = 64
PIX = H * W
CPAD = 66
XPAD = 68
CC = 64
NE = 50

_STATE = {}


def _build_nc():
    import concourse.bass as bass
    import concourse.tile as tile
    from concourse import bacc, mybir

    F32 = mybir.dt.float32
    F16 = mybir.dt.float16
    AF = mybir.ActivationFunctionType
    ALU = mybir.AluOpType

    nc = bacc.Bacc("TRN2", target_bir_lowering=False, debug=False, num_devices=8)

    x2_d = nc.dram_tensor("x2", [2, 128, H, W], F16, kind="ExternalInput")
    wc_d = nc.dram_tensor("wc", [128, 2, CC], F16, kind="ExternalInput")
    bc_d = nc.dram_tensor("bcb", [CC, 1], F32, kind="ExternalInput")
    we_d = nc.dram_tensor("we", [CC, 9, NE], F16, kind="ExternalInput")
    be_d = nc.dram_tensor("beb", [NE, 1], F32, kind="ExternalInput")
    ssum_d = nc.dram_tensor("ssum", [NE, 2], F16, kind="ExternalInput")
    srep_d = nc.dram_tensor("srep", [2, NE], F16, kind="ExternalInput")
    idn_d = nc.dram_tensor("idn", [128, 128], F16, kind="ExternalInput")
    out_d = nc.dram_tensor("out", [2, 128, H, W, 2], F16, kind="ExternalOutput")

    with nc.allow_low_precision(reason="bf16 kernel, tol 2e-2"), \
         tile.TileContext(nc) as tc:
        with (
            tc.tile_pool(name="persist", bufs=1) as pp,
            tc.tile_pool(name="wpool", bufs=1) as wp,
            tc.tile_pool(name="tmp", bufs=2) as tp,
        ):
            # ---- persistent SBUF tensors ----
            xbf = pp.tile([128, 2, XPAD, XPAD], F16, tag="xbf")
            cpad = pp.tile([CC, CPAD, CPAD], F16, tag="cpad")
            expm = pp.tile([NE, PIX], F16, tag="expm")
            rp = [pp.tile([128, H, W, 2], F16, tag=f"rp{b}", name=f"rp{b}")
                  for b in range(2)]

            wc_t = wp.tile([128, 2, CC], F16, tag="wc")
            bc_t = wp.tile([CC, 1], F32, tag="bc")
            we_t = wp.tile([CC, 9, NE], F16, tag="we")
            be_t = wp.tile([NE, 1], F32, tag="be")
            ssum_t = wp.tile([NE, 2], F16, tag="ssum")
            srep_t = wp.tile([2, NE], F16, tag="srep")
            sel_t = wp.tile([NE, NE, 128], F16, tag="sel")
            idn_t = wp.tile([128, 128], F16, tag="idn")

            nc.sync.dma_start(wc_t[:], wc_d.ap()[:])
            nc.sync.dma_start(we_t[:], we_d.ap()[:])
            nc.sync.dma_start(bc_t[:], bc_d.ap()[:])
            nc.sync.dma_start(be_t[:], be_d.ap()[:])
            nc.sync.dma_start(ssum_t[:], ssum_d.ap()[:])
            nc.sync.dma_start(srep_t[:], srep_d.ap()[:])
            nc.sync.dma_start(idn_t[:], idn_d.ap()[:])
            # sel[r, e, c] = idn[r, e] broadcast along c (one ACT copy)
            nc.scalar.copy(
                sel_t[:],
                idn_t[0:NE, 0:NE].rearrange("p (e o) -> p e o", o=1)
                .broadcast_to([NE, NE, 128]))

            # ---- zero pad rings, DMA x into padded interior ----
            for b in range(2):
                t = xbf
                nc.gpsimd.memset(t[:, b, 0:2, :], 0.0)
                nc.gpsimd.memset(t[:, b, XPAD - 2:XPAD, :], 0.0)
                nc.gpsimd.memset(t[:, b, 2:XPAD - 2, 0:2], 0.0)
                nc.gpsimd.memset(t[:, b, 2:XPAD - 2, XPAD - 2:XPAD], 0.0)
                nc.sync.dma_start(t[:, b, 2:2 + H, 2:2 + W], x2_d.ap()[b])
            nc.gpsimd.memset(cpad[:, 0:1, :], 0.0)
            nc.gpsimd.memset(cpad[:, CPAD - 1:CPAD, :], 0.0)
            nc.gpsimd.memset(cpad[:, 1:CPAD - 1, 0:1], 0.0)
            nc.gpsimd.memset(cpad[:, 1:CPAD - 1, CPAD - 1:CPAD], 0.0)

            # ---- B: channel compressor (1x1 conv, K=256 in 2 chunks) ----
            RB = 8  # rows per psum chunk
            psBCD = tc.tile_pool(name="psBCD", bufs=2, space=bass.MemorySpace.PSUM)
            psB = psC = psD = psBCD.__enter__()
            for c in range(H // RB):
                r0 = c * RB
                ps = psB.tile([CC, RB * W], F32, tag="psB")
                for b in range(2):
                    nc.tensor.matmul(
                        ps[:],
                        wc_t[:, b, :],
                        xbf[:, b, 2 + r0:2 + r0 + RB, 2:2 + W],
                        start=(b == 0), stop=(b == 1),
                    )
                nc.scalar.activation(
                    cpad[:, 1 + r0:1 + r0 + RB, 1:1 + W], ps[:], AF.Identity,
                    bias=bc_t[:],
                )

            # ---- C: encoder (3x3 conv) + fused bias+exp ----
            for c in range(H // RB):
                r0 = c * RB
                ps = psC.tile([NE, RB * W], F32, tag="psC")
                for o in range(9):
                    dy, dx = divmod(o, 3)
                    nc.tensor.matmul(
                        ps[:],
                        we_t[:, o, :],
                        cpad[:, r0 + dy:r0 + dy + RB, dx:dx + W],
                        start=(o == 0), stop=(o == 8),
                    )
                nc.scalar.activation(
                    expm[:, r0 * W:(r0 + RB) * W], ps[:], AF.Exp,
                    bias=be_t[:],
                )

            # ---- D: softmax denominators and normalization ----
            DC = 512
            for c in range(PIX // DC):
                sl = slice(c * DC, (c + 1) * DC)
                ps = psD.tile([2, DC], F32, tag="psS")
                nc.tensor.matmul(ps[:], ssum_t[:], expm[:, sl],
                                 start=True, stop=True)
                rc = tp.tile([2, DC], F16, tag="recip", name="recip")
                nc.vector.reciprocal(rc[:], ps[:])
                pr = psD.tile([NE, DC], F32, tag="psR")
                nc.tensor.matmul(pr[:], srep_t[:], rc[:],
                                 start=True, stop=True)
                nc.vector.tensor_tensor(expm[:, sl], expm[:, sl], pr[:], op=ALU.mult)
            psBCD.__exit__(None, None, None)

            # ---- E: reassembly ----
            # per (pixel chunk, j): broadcast mask row k (selector matmul,
            # f32r), evict to bf16, multiply both channel-halves of bf16
            # padded x in one DVE op, accumulate the 25 tap products in
            # PSUM with identity matmuls (f32 accumulate on PE).
            psE_cm = tc.tile_pool(name="psE", bufs=2, space=bass.MemorySpace.PSUM)
            psE = psE_cm.__enter__()
            psA_cm = tc.tile_pool(name="psA", bufs=1, space=bass.MemorySpace.PSUM)
            psA = psA_cm.__enter__()
            RE = 16  # rows per chunk (1024 px)
            for ch in range(H // RE):
                r0 = ch * RE
                for j in range(2):
                    accP = [psA.tile([128, RE * W], F32, tag=f"accP{b}",
                                     name=f"accP{b}") for b in range(2)]
                    for k in range(25):
                        dy, dx = divmod(k, 5)
                        e = k * 2 + j
                        ps = psE.tile([128, RE * W], F32, tag="psE")
                        for s in range(RE * W // 512):
                            nc.tensor.matmul(
                                ps[:, s * 512:(s + 1) * 512],
                                sel_t[:, e, :],
                                expm[:, r0 * W + s * 512:r0 * W + (s + 1) * 512],
                                start=True, stop=True,
                            )
                        msk = tp.tile([128, 1, RE, W], F16, tag="msk", name="msk")
                        nc.scalar.copy(msk[:], ps[:])
                        prod = tp.tile([128, 2, RE, W], F16, tag="prod",
                                       name="prod")
                        nc.vector.tensor_tensor(
                            prod[:],
                            xbf[:, :, r0 + dy:r0 + dy + RE, dx:dx + W],
                            msk[:].broadcast_to([128, 2, RE, W]),
                            op=ALU.mult)
                        for b in range(2):
                            for s in range(RE * W // 512):
                                rs = s * 512 // W
                                nc.tensor.matmul(
                                    accP[b][:, s * 512:(s + 1) * 512],
                                    idn_t[:],
                                    prod[:, b, rs:rs + 512 // W, :],
                                    start=(k == 0), stop=(k == 24),
                                )
                    for b in range(2):
                        nc.scalar.copy(rp[b][:, r0:r0 + RE, :, j], accP[b][:])
            psA_cm.__exit__(None, None, None)
            psE_cm.__exit__(None, None, None)

            # ---- F: store ----
            for b in range(2):
                nc.sync.dma_start(out_d.ap()[b], rp[b][:])

    nc.compile()
    return nc




def _get_runner():
    if "runner" in _STATE:
        return _STATE["runner"]

    import jax
    from jax.sharding import Mesh, PartitionSpec
    from jax.experimental.shard_map import shard_map
    from concourse import mybir
    from concourse.bass2jax import (_bass_exec_p, install_neuronx_cc_hook,
                                    partition_id_tensor)

    nc = _build_nc()
    _STATE["nc"] = nc
    install_neuronx_cc_hook()

    part_name = (nc.partition_id_tensor.name
                 if nc.partition_id_tensor else None)
    in_names, out_names, out_avals, zero_outs = [], [], [], []
    for alloc in nc.m.functions[0].allocations:
        if not isinstance(alloc, mybir.MemoryLocationSet):
            continue
        name = alloc.memorylocations[0].name
        if alloc.kind == "ExternalInput":
            if name != part_name:
                in_names.append(name)
        elif alloc.kind == "ExternalOutput":
            out_names.append(name)
            shape = tuple(alloc.tensor_shape)
            dtype = mybir.dt.np(alloc.dtype)
            out_avals.append(jax.core.ShapedArray(shape, dtype))
            zero_outs.append(np.zeros(shape, dtype))
    n_params = len(in_names)
    n_outs = len(out_names)
    all_names = in_names + out_names
    if part_name is not None:
        all_names = all_names + [part_name]
    donate = tuple(range(n_params, n_params + n_outs))

    def _body(*args):
        operands = list(args)
        if part_name is not None:
            operands.append(partition_id_tensor())
        outs = _bass_exec_p.bind(
            *operands,
            out_avals=tuple(out_avals),
            in_names=tuple(all_names),
            out_names=tuple(out_names),
            lowering_input_output_aliases=(),
            sim_require_finite=True,
            sim_require_nnan=True,
            nc=nc,
        )
        return tuple(outs)

    devices = jax.devices()[:8]
    mesh = Mesh(np.asarray(devices), ("core",))
    specs = (PartitionSpec("core"),) * (n_params + n_outs)
    sharded = jax.jit(
        shard_map(_body, mesh=mesh, in_specs=specs,
                  out_specs=(PartitionSpec("core"),) * n_outs,
                  check_rep=False),
        donate_argnums=donate, keep_unused=True)

    runner = (sharded, in_names, out_names, out_avals, zero_outs)
    _STATE["runner"] = runner
    return runner


def _prep_in_maps(x, Wc, bc, We, be):
    """Per-core input maps. x:[4,256,64,64] Wc:[64,256,1,1] We:[100,64,3,3]."""
    BF = np.float16
    x = np.ascontiguousarray(x, np.float32).astype(BF)
    Wc2 = np.ascontiguousarray(Wc[:, :, 0, 0], np.float32)      # [64, 256]
    wc_arr = np.stack([np.ascontiguousarray(Wc2[:, i * 128:(i + 1) * 128].T)
                       for i in range(2)], axis=1)               # [128,2,64]
    wc_arr = np.ascontiguousarray(wc_arr).astype(BF)
    bc_arr = np.ascontiguousarray(bc.reshape(CC, 1), np.float32)
    ssum_arr = np.zeros((NE, 2), BF)
    ssum_arr[np.arange(NE), np.arange(NE) % 2] = 1.0
    srep_arr = np.ascontiguousarray(ssum_arr.T)
    idn_arr = np.eye(128, dtype=BF)

    in_maps = []
    for core in range(8):
        n, i = divmod(core, 2)
        rows = (4 * np.arange(25)[:, None] + 2 * i +
                np.arange(2)[None, :]).reshape(-1)               # e = 4k+2i+j
        we_sub = We[rows]                                        # [50,64,3,3]
        we_arr = np.stack([np.ascontiguousarray(we_sub[:, :, o // 3, o % 3].T)
                           for o in range(9)], axis=1).astype(BF)  # [64,9,50]
        be_arr = np.ascontiguousarray(be[rows].reshape(NE, 1), np.float32)
        in_maps.append({
            "x2": np.ascontiguousarray(x[n].reshape(2, 128, H, W)),
            "wc": wc_arr, "bcb": bc_arr,
            "we": np.ascontiguousarray(we_arr), "beb": be_arr,
            "ssum": ssum_arr, "srep": srep_arr, "idn": idn_arr,
        })
    return in_maps




def kernel(x, Wc, bc, We, be):
    sharded, in_names, out_names, out_avals, zero_outs = _get_runner()
    in_maps = _prep_in_maps(x, Wc, bc, We, be)

    concat_in = [np.concatenate([in_maps[c][nm] for c in range(8)], axis=0)
                 for nm in in_names]
    concat_zeros = [np.zeros((8 * z.shape[0], *z.shape[1:]), z.dtype)
                    for z in zero_outs]
    out_arrs = sharded(*concat_in, *concat_zeros)

    oi = out_names.index("out")
    per_core = np.asarray(out_arrs[oi]).reshape(8, *out_avals[oi].shape)

    full = np.empty((4, 256, 2 * H, 2 * W), np.float32)
    for core in range(8):
        n, i = divmod(core, 2)
        o = per_core[core].astype(np.float32).reshape(256, H, 2 * W)
        full[n, :, i::2, :] = o
    return full


# revision 6
# speedup vs baseline: 1.2924x; 1.2924x over previous
"""CARAFE upsample (nn_CARAFEPack) on 8 TRN2 NeuronCores via a Bass/Tile kernel.

Full inputs in, full output out. Sharding: core = (n, i): image n = core//2,
output-row parity i = core%2; each core produces out[n, :, i::2, :].

On-device per core: channel-compressor 1x1 conv and encoder 3x3 conv as
PSUM-accumulated f16 matmuls with fused bias (+exp) on the scalar engine;
softmax normalization via 0/1-matrix matmuls + reciprocal; CARAFE
reassembly with per-(j,tap) selector-matmul mask broadcasts, f16
evictions, merged DVE multiplies over shifted padded-x windows, and tap
accumulation in PSUM f32 via identity matmuls. Output f16.

The compiled executable (jax custom call wrapping the Bass NEFF, sharded
over 8 cores) is cached at module level so repeat kernel() calls skip
re-tracing.
"""
import sys

sys.path.insert(0, '/opt/trn_rl_repo')

import numpy as np

H = W = 64
PIX = H * W
CPAD = 66
XPAD = 68
CC = 64
NE = 50

_STATE = {}


def _build_nc():
    import concourse.bass as bass
    import concourse.tile as tile
    from concourse import bacc, mybir

    F32 = mybir.dt.float32
    F16 = mybir.dt.float16
    AF = mybir.ActivationFunctionType
    ALU = mybir.AluOpType

    nc = bacc.Bacc("TRN2", target_bir_lowering=False, debug=False, num_devices=8)

    x2_d = nc.dram_tensor("x2", [2, 128, H, W], F16, kind="ExternalInput")
    wc_d = nc.dram_tensor("wc", [128, 2, CC], F16, kind="ExternalInput")
    bc_d = nc.dram_tensor("bcb", [CC, 1], F32, kind="ExternalInput")
    we_d = nc.dram_tensor("we", [CC, 9, NE], F16, kind="ExternalInput")
    be_d = nc.dram_tensor("beb", [NE, 1], F32, kind="ExternalInput")
    ssum_d = nc.dram_tensor("ssum", [NE, 2], F16, kind="ExternalInput")
    srep_d = nc.dram_tensor("srep", [2, NE], F16, kind="ExternalInput")
    idn_d = nc.dram_tensor("idn", [128, 128], F16, kind="ExternalInput")
    out_d = nc.dram_tensor("out", [2, 128, H, W, 2], F16, kind="ExternalOutput")

    with nc.allow_low_precision(reason="bf16 kernel, tol 2e-2"), \
         tile.TileContext(nc) as tc:
        with (
            tc.tile_pool(name="persist", bufs=1) as pp,
            tc.tile_pool(name="wpool", bufs=1) as wp,
            tc.tile_pool(name="tmp", bufs=2) as tp,
        ):
            # ---- persistent SBUF tensors ----
            xbf = pp.tile([128, 2, XPAD, XPAD], F16, tag="xbf")
            cpad = pp.tile([CC, CPAD, CPAD], F16, tag="cpad")
            expm = pp.tile([NE, PIX], F16, tag="expm")
            rp = [pp.tile([128, H, W, 2], F16, tag=f"rp{b}", name=f"rp{b}")
                  for b in range(2)]

            wc_t = wp.tile([128, 2, CC], F16, tag="wc")
            bc_t = wp.tile([CC, 1], F32, tag="bc")
            we_t = wp.tile([CC, 9, NE], F16, tag="we")
            be_t = wp.tile([NE, 1], F32, tag="be")
            ssum_t = wp.tile([NE, 2], F16, tag="ssum")
            srep_t = wp.tile([2, NE], F16, tag="srep")
            sel_t = wp.tile([NE, NE, 128], F16, tag="sel")
            idn_t = wp.tile([128, 128], F16, tag="idn")

            nc.sync.dma_start(wc_t[:], wc_d.ap()[:])
            nc.sync.dma_start(we_t[:], we_d.ap()[:])
            nc.sync.dma_start(bc_t[:], bc_d.ap()[:])
            nc.sync.dma_start(be_t[:], be_d.ap()[:])
            nc.sync.dma_start(ssum_t[:], ssum_d.ap()[:])
            nc.sync.dma_start(srep_t[:], srep_d.ap()[:])
            nc.sync.dma_start(idn_t[:], idn_d.ap()[:])
            # sel[r, e, c] = idn[r, e] broadcast along c (one ACT copy)
            nc.scalar.copy(
                sel_t[:],
                idn_t[0:NE, 0:NE].rearrange("p (e o) -> p e o", o=1)
                .broadcast_to([NE, NE, 128]))

            # ---- zero pad rings, DMA x into padded interior ----
            for b in range(2):
                t = xbf
                nc.gpsimd.memset(t[:, b, 0:2, :], 0.0)
                nc.gpsimd.memset(t[:, b, XPAD - 2:XPAD, :], 0.0)
                nc.gpsimd.memset(t[:, b, 2:XPAD - 2, 0:2], 0.0)
                nc.gpsimd.memset(t[:, b, 2:XPAD - 2, XPAD - 2:XPAD], 0.0)
                nc.sync.dma_start(t[:, b, 2:2 + H, 2:2 + W], x2_d.ap()[b])
            nc.gpsimd.memset(cpad[:, 0:1, :], 0.0)
            nc.gpsimd.memset(cpad[:, CPAD - 1:CPAD, :], 0.0)
            nc.gpsimd.memset(cpad[:, 1:CPAD - 1, 0:1], 0.0)
            nc.gpsimd.memset(cpad[:, 1:CPAD - 1, CPAD - 1:CPAD], 0.0)

            # ---- B: channel compressor (1x1 conv, K=256 in 2 chunks) ----
            RB = 8  # rows per psum chunk
            psBCD = tc.tile_pool(name="psBCD", bufs=2, space=bass.MemorySpace.PSUM)
            psB = psC = psD = psBCD.__enter__()
            for c in range(H // RB):
                r0 = c * RB
                ps = psB.tile([CC, RB * W], F32, tag="psB")
                for b in range(2):
                    nc.tensor.matmul(
                        ps[:],
                        wc_t[:, b, :],
                        xbf[:, b, 2 + r0:2 + r0 + RB, 2:2 + W],
                        start=(b == 0), stop=(b == 1),
                    )
                nc.scalar.activation(
                    cpad[:, 1 + r0:1 + r0 + RB, 1:1 + W], ps[:], AF.Identity,
                    bias=bc_t[:],
                )

            # ---- C: encoder (3x3 conv) + fused bias+exp ----
            for c in range(H // RB):
                r0 = c * RB
                ps = psC.tile([NE, RB * W], F32, tag="psC")
                for o in range(9):
                    dy, dx = divmod(o, 3)
                    nc.tensor.matmul(
                        ps[:],
                        we_t[:, o, :],
                        cpad[:, r0 + dy:r0 + dy + RB, dx:dx + W],
                        start=(o == 0), stop=(o == 8),
                    )
                nc.scalar.activation(
                    expm[:, r0 * W:(r0 + RB) * W], ps[:], AF.Exp,
                    bias=be_t[:],
                )

            # ---- D: softmax denominators and normalization ----
            DC = 512
            for c in range(PIX // DC):
                sl = slice(c * DC, (c + 1) * DC)
                ps = psD.tile([2, DC], F32, tag="psS")
                nc.tensor.matmul(ps[:], ssum_t[:], expm[:, sl],
                                 start=True, stop=True)
                rc = tp.tile([2, DC], F16, tag="recip", name="recip")
                nc.vector.reciprocal(rc[:], ps[:])
                pr = psD.tile([NE, DC], F32, tag="psR")
                nc.tensor.matmul(pr[:], srep_t[:], rc[:],
                                 start=True, stop=True)
                nc.vector.tensor_tensor(expm[:, sl], expm[:, sl], pr[:], op=ALU.mult)
            psBCD.__exit__(None, None, None)

            # ---- E: reassembly ----
            # per (pixel chunk, j): broadcast mask row k (selector matmul,
            # f32r), evict to bf16, multiply both channel-halves of bf16
            # padded x in one DVE op, accumulate the 25 tap products in
            # PSUM with identity matmuls (f32 accumulate on PE).
            psE_cm = tc.tile_pool(name="psE", bufs=2, space=bass.MemorySpace.PSUM)
            psE = psE_cm.__enter__()
            psA_cm = tc.tile_pool(name="psA", bufs=1, space=bass.MemorySpace.PSUM)
            psA = psA_cm.__enter__()
            RE = 16  # rows per chunk (1024 px)
            for ch in range(H // RE):
                r0 = ch * RE
                for j in range(2):
                    accP = [psA.tile([128, RE * W], F32, tag=f"accP{b}",
                                     name=f"accP{b}") for b in range(2)]
                    for k in range(25):
                        dy, dx = divmod(k, 5)
                        e = k * 2 + j
                        ps = psE.tile([128, RE * W], F32, tag="psE")
                        for s in range(RE * W // 512):
                            nc.tensor.matmul(
                                ps[:, s * 512:(s + 1) * 512],
                                sel_t[:, e, :],
                                expm[:, r0 * W + s * 512:r0 * W + (s + 1) * 512],
                                start=True, stop=True,
                            )
                        msk = tp.tile([128, 1, RE, W], F16, tag="msk", name="msk")
                        nc.scalar.copy(msk[:], ps[:])
                        prod = tp.tile([128, 2, RE, W], F16, tag="prod",
                                       name="prod")
                        nc.vector.tensor_tensor(
                            prod[:],
                            xbf[:, :, r0 + dy:r0 + dy + RE, dx:dx + W],
                            msk[:].broadcast_to([128, 2, RE, W]),
                            op=ALU.mult)
                        for b in range(2):
                            for s in range(RE * W // 512):
                                rs = s * 512 // W
                                nc.tensor.matmul(
                                    accP[b][:, s * 512:(s + 1) * 512],
                                    idn_t[:],
                                    prod[:, b, rs:rs + 512 // W, :],
                                    start=(k == 0), stop=(k == 24),
                                )
                    for b in range(2):
                        nc.scalar.copy(rp[b][:, r0:r0 + RE, :, j], accP[b][:])
            psA_cm.__exit__(None, None, None)
            psE_cm.__exit__(None, None, None)

            # ---- F: store ----
            for b in range(2):
                nc.sync.dma_start(out_d.ap()[b], rp[b][:])

    nc.compile()
    return nc




def _get_runner():
    if "runner" in _STATE:
        return _STATE["runner"]

    import jax
    from jax.sharding import Mesh, PartitionSpec
    from jax.experimental.shard_map import shard_map
    from concourse import mybir
    from concourse.bass2jax import (_bass_exec_p, install_neuronx_cc_hook,
                                    partition_id_tensor)

    nc = _build_nc()
    _STATE["nc"] = nc
    install_neuronx_cc_hook()

    part_name = (nc.partition_id_tensor.name
                 if nc.partition_id_tensor else None)
    in_names, out_names, out_avals, zero_outs = [], [], [], []
    for alloc in nc.m.functions[0].allocations:
        if not isinstance(alloc, mybir.MemoryLocationSet):
            continue
        name = alloc.memorylocations[0].name
        if alloc.kind == "ExternalInput":
            if name != part_name:
                in_names.append(name)
        elif alloc.kind == "ExternalOutput":
            out_names.append(name)
            shape = tuple(alloc.tensor_shape)
            dtype = mybir.dt.np(alloc.dtype)
            out_avals.append(jax.core.ShapedArray(shape, dtype))
            zero_outs.append(np.zeros(shape, dtype))
    n_params = len(in_names)
    n_outs = len(out_names)
    all_names = in_names + out_names
    if part_name is not None:
        all_names = all_names + [part_name]

    def _body(*args):
        operands = list(args)
        if part_name is not None:
            operands.append(partition_id_tensor())
        outs = _bass_exec_p.bind(
            *operands,
            out_avals=tuple(out_avals),
            in_names=tuple(all_names),
            out_names=tuple(out_names),
            lowering_input_output_aliases=(),
            sim_require_finite=True,
            sim_require_nnan=True,
            nc=nc,
        )
        return tuple(outs)

    devices = jax.devices()[:8]
    mesh = Mesh(np.asarray(devices), ("core",))
    specs = (PartitionSpec("core"),) * (n_params + n_outs)
    sharded = jax.jit(
        shard_map(_body, mesh=mesh, in_specs=specs,
                  out_specs=(PartitionSpec("core"),) * n_outs,
                  check_rep=False),
        keep_unused=True)

    # device-resident zero output buffers, uploaded once and reused (the
    # kernel writes every output element, so no per-call re-zeroing needed)
    from jax.sharding import NamedSharding
    zeros_dev = [
        jax.device_put(np.zeros((8 * z.shape[0], *z.shape[1:]), z.dtype),
                       NamedSharding(mesh, PartitionSpec("core")))
        for z in zero_outs
    ]
    runner = (sharded, in_names, out_names, out_avals, zeros_dev)
    _STATE["runner"] = runner
    return runner


def _prep_in_maps(x, Wc, bc, We, be):
    """Per-core input maps. x:[4,256,64,64] Wc:[64,256,1,1] We:[100,64,3,3]."""
    BF = np.float16
    x = np.ascontiguousarray(x, np.float32).astype(BF)
    Wc2 = np.ascontiguousarray(Wc[:, :, 0, 0], np.float32)      # [64, 256]
    wc_arr = np.stack([np.ascontiguousarray(Wc2[:, i * 128:(i + 1) * 128].T)
                       for i in range(2)], axis=1)               # [128,2,64]
    wc_arr = np.ascontiguousarray(wc_arr).astype(BF)
    bc_arr = np.ascontiguousarray(bc.reshape(CC, 1), np.float32)
    ssum_arr = np.zeros((NE, 2), BF)
    ssum_arr[np.arange(NE), np.arange(NE) % 2] = 1.0
    srep_arr = np.ascontiguousarray(ssum_arr.T)
    idn_arr = np.eye(128, dtype=BF)

    in_maps = []
    for core in range(8):
        n, i = divmod(core, 2)
        rows = (4 * np.arange(25)[:, None] + 2 * i +
                np.arange(2)[None, :]).reshape(-1)               # e = 4k+2i+j
        we_sub = We[rows]                                        # [50,64,3,3]
        we_arr = np.stack([np.ascontiguousarray(we_sub[:, :, o // 3, o % 3].T)
                           for o in range(9)], axis=1).astype(BF)  # [64,9,50]
        be_arr = np.ascontiguousarray(be[rows].reshape(NE, 1), np.float32)
        in_maps.append({
            "x2": np.ascontiguousarray(x[n].reshape(2, 128, H, W)),
            "wc": wc_arr, "bcb": bc_arr,
            "we": np.ascontiguousarray(we_arr), "beb": be_arr,
            "ssum": ssum_arr, "srep": srep_arr, "idn": idn_arr,
        })
    return in_maps




def kernel(x, Wc, bc, We, be):
    sharded, in_names, out_names, out_avals, zeros_dev = _get_runner()
    in_maps = _prep_in_maps(x, Wc, bc, We, be)

    concat_in = [np.concatenate([in_maps[c][nm] for c in range(8)], axis=0)
                 for nm in in_names]
    out_arrs = sharded(*concat_in, *zeros_dev)

    oi = out_names.index("out")
    per_core = np.asarray(out_arrs[oi]).reshape(8, *out_avals[oi].shape)

    full = np.empty((4, 256, 2 * H, 2 * W), np.float32)
    for core in range(8):
        n, i = divmod(core, 2)
        o = per_core[core].astype(np.float32).reshape(256, H, 2 * W)
        full[n, :, i::2, :] = o
    return full
